# revision 26
# baseline (speedup 1.0000x reference)
"""Trainium2 Bass kernel for nn_DeepPatchEncoder.

Math: the reference collapses to
    out[b] = A_X[b] @ W_dense + D_const
    D_const = (A_P + W_emb) @ W_dense + b_dense
where A_X[b] is the coarse-patchify permutation of X[b] and A_P is a
permutation of the conv-branch output (conv3x3 s2 on W_emb viewed as a
[32,32,1024] image, then BN + LeakyReLU).

Sharding (zero cross-core communication):
  Core k computes output ROWS n0 in [128k, 128(k+1)) for ALL 8 batches.
  Those rows of D_const need exactly conv output channels [512k, 512k+512),
  which core k computes itself.

Precision strategy: the conv branch and D_const are SMALL additive terms
(|pos2| ~ 0.05 vs |X-patch| ~ 1), so they run in fp8e4m3 with
perf_mode=DoubleRow (2 fp8 weights per PE cell, K=256 per matmul) --
~2x tensor throughput and half the DMA bytes for conv weights.  The
batched main matmul A_X @ W_dense stays fp16 (fp8 would blow the 2e-2
error budget).  Scales (all powers of 2, folded on host):
    pe8 = 64*pe, cw8 = 64*(conv_w*bn_scale)  -> conv PSUM = 4096*y
    Y2  = 8*lrelu(y)   (copy-out scale 1/512; lrelu commutes with +scale)
    wrows = 8*W_emb^T, lhsC8 = Y2 + wrows = 8*lhsC
    wd8 = 32*W_dense   -> D PSUM = 256*D, drows = PSUM/256
Outputs written fp16 (quantum ~2e-3 << 0.068 abs budget), upcast on host.

Schedule: conv data (pe/cw) rides the sync HWDGE ring FIRST and alone
(the ring saturates ~330 GB/s and the conv consumes it at ~408 GB/s,
so any filler stretches the conv window 1:1); tiny tensors (t2, ones,
wrows, bd) ride the scalar ring in parallel so the first conv matmul
only waits on pe0+cw0.  PE warm-up matmuls on memset data run before
the first data matmul so HAM reaches K=8/8.  Post-conv sync ring:
axt0, axt1, wd0-7, axt2, axt3, wd8, axt4-7.  Main-phase matmuls are
emitted kt-skewed across the first 4 PSUM groups (b0/b1) to follow the
wd stream, then b2/b3, then the D matmuls (their wd8 lands about then;
they fill the axt4 wait), then b4-b7.  drows runs on GpSimd so the
in-order Vector queue (output adds) can't deadlock on it; output DMAs
ride the scalar HWDGE ring.
"""

import numpy as np
import ml_dtypes

B = 8
NC = 8
IMG = 1024
N0 = 1024
D0 = 1024
BN_EPS = 1e-3
ALPHA = 0.3

SA = 64.0     # pe scale
SB = 64.0     # cw scale
SC = SA * SB  # conv psum scale = 4096
SY = 8.0      # Y2 / wrows / lhsC scale
SW8 = 32.0    # wd8 scale
SD = SY * SW8 # D psum scale = 256

_CACHE = {}

F8 = ml_dtypes.float8_e4m3


def _f8(x):
    return np.clip(x, -240.0, 240.0).astype(F8)


# ---------------------------------------------------------------- host prep

def _perms():
    # rho2: lhsC position d0'' = 128*k2 + p2 -> natural d0 = 512u+32a+16v+bh
    #   with k2 = 4u + 2v + mb, a = 8*mb + p2//16, bh = p2 % 16
    i = np.arange(1024)
    k2, p2 = i // 128, i % 128
    u, v, mb = k2 // 4, (k2 // 2) % 2, k2 % 2
    a, bh = 8 * mb + p2 // 16, p2 % 16
    rho = 512 * u + 32 * a + 16 * v + bh
    # sigma2: conv rhs column pos2 = 128*(2u+v) + 32*ocb + j0 -> in-shard
    #   channel c_loc = 128*ocb + 64u + 2j0 + v  (so lhsC slices are plain
    #   free-dim slices of the conv output)
    p = np.arange(512)
    uv, ocb, j0 = p // 128, (p // 32) % 4, p % 32
    uu, vv = uv // 2, uv % 2
    sigma = 128 * ocb + 64 * uu + 2 * j0 + vv
    return rho, sigma


def host_prep(inputs):
    f16 = np.float16
    X = np.asarray(inputs["X"], np.float32).reshape(B, IMG, IMG)
    W_emb = np.asarray(inputs["W_emb"], np.float32)
    conv_w = np.asarray(inputs["conv_w"], np.float32)
    conv_b = np.asarray(inputs["conv_b"], np.float32)
    g = np.asarray(inputs["bn_gamma"], np.float32)
    be = np.asarray(inputs["bn_beta"], np.float32)
    mu = np.asarray(inputs["bn_mean"], np.float32)
    var = np.asarray(inputs["bn_var"], np.float32)
    W_dense = np.asarray(inputs["W_dense"], np.float32)
    b_dense = np.asarray(inputs["b_dense"], np.float32)

    rho, sigma = _perms()

    s_all = g / np.sqrt(var + BN_EPS)
    t_all = (conv_b - mu) * s_all + be

    # pe lhsT blocks, per (dd, tp=kt//2, i=kt%2, mb):
    # pe[dd, p, tp, i, mb, 16a'+oj] = peT[128*(2tp+i)+p, di+2*(8mb+a'), dj+2oj]
    peT = np.zeros((N0, 33, 33), np.float32)
    peT[:, :32, :32] = W_emb.reshape(N0, 32, 32)
    peb = np.empty((9, N0, 256), np.float32)
    for dd in range(9):
        di, dj = dd // 3, dd % 3
        blk = peT[:, di:di + 31:2, dj:dj + 31:2]       # [1024, 16, 16]
        peb[dd] = blk.reshape(N0, 256)                 # 16a+oj: a = 8mb+a'
    # free dim (a, oj) == (mb, a', oj); channel 1024 -> (kt-major, p)
    peb = peb.reshape(9, 8, 128, 2, 128).transpose(0, 2, 1, 3, 4)
    pe_host = _f8(peb.reshape(9, 128, 4, 2, 2, 128) * SA)

    # W_dense row-permuted by rho, K-tile major [8, 128, 1024] fp16
    wd_perm = W_dense[rho, :]                                    # [1024, 1024]
    wd_host = np.ascontiguousarray(wd_perm.reshape(8, 128, D0)).astype(f16)
    # fp8 copy for the D matmul, DoubleRow pair layout
    # wd8[tp, p, i, nb, n] = SW8 * W_dense[rho[128*(2tp+i)+p], 512*nb+n]
    wd8_host = _f8(
        (wd_perm * SW8).reshape(4, 2, 128, 2, 512).transpose(0, 2, 1, 3, 4)
    )

    bd_host = (b_dense * SD).reshape(1, D0).astype(f16)

    in_maps = []
    for k in range(NC):
        ch = 512 * k + sigma
        # conv weights as DoubleRow matmul RHS, BN scale s folded in:
        # cw[dd, p, tp, i, oc] = SB * s[ch[oc]] * conv_w[dd, 128*(2tp+i)+p, ch[oc]]
        cws = conv_w[:, :, :, ch] * s_all[ch]          # [3,3,1024,512]
        cw = cws.reshape(9, 8, 128, 512).transpose(0, 2, 1, 3)
        cw_host = _f8(cw.reshape(9, 128, 4, 2, 512) * SB)
        # BN shift as a K=1 seed-matmul rhs row, in PSUM scale
        t2 = (t_all[ch] * SC).reshape(1, 512).astype(f16)
        # A_X^T for this core's row strip, rho-permuted, SBUF layout
        # [8b, 128part, 8kt*128j]: axt[b, p, 128*kt+j] = A_X^T[b, 128kt+p, j]
        Xs = X[:, 128 * k:128 * (k + 1), :]        # [8,128,1024]
        axt = Xs.reshape(B, 4, 32, 32, 32).transpose(0, 2, 4, 1, 3).reshape(B, 1024, 128)
        axt = axt[:, rho, :].reshape(B, 8, 128, 128).transpose(0, 2, 1, 3)
        axt = np.ascontiguousarray(axt.reshape(B, 128, 1024)).astype(f16)
        # W_emb row block transposed into lhsC layout, pre-scaled by SY:
        # wrows[p, 128*k2 + j] = SY * W_emb[128k + j, rho[128*k2 + p]]
        wr = W_emb[128 * k:128 * (k + 1), :][:, rho]        # [128j, 1024d0']
        wrows = np.ascontiguousarray(
            (wr.T * SY).reshape(8, 128, 128).transpose(1, 0, 2).reshape(128, 4, 2, 128)
        ).astype(f16)
        in_maps.append({
            "cw": cw_host, "pe": pe_host, "t2": t2, "axt": axt,
            "wrows": wrows, "wd": wd_host, "wd8": wd8_host, "bd": bd_host,
            "ones": np.ones((1, 128), np.float16),
        })
    return in_maps


# ---------------------------------------------------------------- device code

def _build():
    import concourse.tile as tile
    import concourse.mybir as mybir
    from concourse import bacc

    f32 = mybir.dt.float32
    f16 = mybir.dt.float16
    f8 = mybir.dt.float8e4
    Alu = mybir.AluOpType
    DR = mybir.MatmulPerfMode.DoubleRow

    nc = bacc.Bacc("TRN2", target_bir_lowering=False, debug=False)

    cw_d = nc.dram_tensor("cw", [9, 128, 4, 2, 512], f8, kind="ExternalInput").ap()
    pe_d = nc.dram_tensor("pe", [9, 128, 4, 2, 2, 128], f8, kind="ExternalInput").ap()
    t2_d = nc.dram_tensor("t2", [1, 512], f16, kind="ExternalInput").ap()
    axt_d = nc.dram_tensor("axt", [8, 128, 1024], f16, kind="ExternalInput").ap()
    wrows_d = nc.dram_tensor("wrows", [128, 4, 2, 128], f16, kind="ExternalInput").ap()
    wd_d = nc.dram_tensor("wd", [8, 128, 1024], f16, kind="ExternalInput").ap()
    wd8_d = nc.dram_tensor("wd8", [4, 128, 2, 2, 512], f8, kind="ExternalInput").ap()
    bd_d = nc.dram_tensor("bd", [1, 1024], f16, kind="ExternalInput").ap()
    ones_d = nc.dram_tensor("ones", [1, 128], f16, kind="ExternalInput").ap()
    out_d = nc.dram_tensor("out", [8, 128, 1024], f16, kind="ExternalOutput").ap()

    with tile.TileContext(nc) as tc:
        with (
            tc.tile_pool(name="kpool", bufs=1) as kpool,
            tc.tile_pool(name="zpool", bufs=2) as zpool,
            tc.tile_pool(name="opool", bufs=8) as opool,
        ):
            # persistent SBUF tensors
            pe_sb = kpool.tile([128, 9, 4, 2, 2, 128], f8, tag="pe")
            cw_sb = kpool.tile([128, 9, 4, 2, 512], f8, tag="cw")
            t2_sb = kpool.tile([1, 512], f16, tag="t2")
            Y2 = kpool.tile([128, 2, 512], f16, tag="Y2")   # 8*lrelu(BN(conv))
            lhsC8 = kpool.tile([128, 4, 2, 128], f8, tag="lhsC")
            drows = kpool.tile([128, 1024], f16, tag="drows")
            wrows_sb = kpool.tile([128, 4, 2, 128], f16, tag="wrows")
            bd_sb = kpool.tile([1, 1024], f16, tag="bd")
            ones_sb = kpool.tile([1, 128], f16, tag="ones")
            wd_sb = kpool.tile([128, 8, 1024], f16, tag="wd")
            wd8_sb = kpool.tile([128, 4, 2, 2, 512], f8, tag="wd8")
            axt_sb = kpool.tile([128, 8, 1024], f16, tag="axt")

            # ---------------- conv with BN folded into the PE:
            #   scale s folded into cw on host; shift t is a K=1 seed
            #   matmul (ones^T @ t2) accumulated first into each PSUM bank.
            # DoubleRow fp8: lhsT = pe [128ch, 2kt, 128m], rhs = cw
            # [128ch, 2kt, 512oc]; each MM contracts K=256.
            with tc.tile_pool(name="psA", bufs=1, space="PSUM") as psA, \
                 tc.tile_pool(name="psW", bufs=1, space="PSUM") as psW:
                y2ps = [psA.tile([128, 512], f32, tag=f"y2{mb}", name=f"y2ps{mb}")
                        for mb in range(2)]
                # t2/ones lead the sync ring (tiny) so the seeds run just
                # before the first conv MM
                nc.sync.dma_start(t2_sb[:], t2_d[:])
                nc.sync.dma_start(ones_sb[:], ones_d[:])
                # PE warm-up on memset data (no DMA deps; gpsimd memsets run
                # earliest): HAM hits K=8/8 before the first data matmul
                wps = psW.tile([128, 512], f32, tag="wps")
                wsrc = kpool.tile([1, 128], f16, tag="wsrc")
                wpad = kpool.tile([1, 512], f16, tag="wpad")
                nc.gpsimd.memset(wsrc[:], 1.0)
                nc.gpsimd.memset(wpad[:], 0.0)
                for w in range(6):
                    nc.tensor.matmul(
                        wps[:], wsrc[:], wpad[:],
                        start=(w == 0), stop=(w == 5))
                for mb in range(2):
                    nc.tensor.matmul(
                        y2ps[mb][:], ones_sb[:], t2_sb[:],
                        start=True, stop=False)
                # conv data split across BOTH HWDGE rings, balanced per
                # round (each ring carries one tp-half of pe AND cw): the
                # SDMA engines round-robin across queues with pending work,
                # so the pair arrives at aggregate HBM rate (~376 GB/s
                # measured) instead of one ring's ~334.
                for dd in range(9):
                    nc.sync.dma_start(pe_sb[:, dd, :2], pe_d[dd][:, :2])
                    nc.sync.dma_start(cw_sb[:, dd, :2], cw_d[dd][:, :2])
                    nc.scalar.dma_start(pe_sb[:, dd, 2:], pe_d[dd][:, 2:])
                    nc.scalar.dma_start(cw_sb[:, dd, 2:], cw_d[dd][:, 2:])
                    for mb in range(2):
                        for tp in range(4):
                            nc.tensor.matmul(
                                y2ps[mb][:],
                                pe_sb[:, dd, tp, :, mb, :],
                                cw_sb[:, dd, tp, :, :],
                                start=False,
                                stop=(dd == 8 and tp == 3 and mb == 1),
                                perf_mode=DR)
                # dense-path loads, in consumption order
                nc.scalar.dma_start(wrows_sb[:], wrows_d[:])
                nc.scalar.dma_start(bd_sb[:], bd_d[:])
                nc.sync.dma_start(axt_sb[:, 0], axt_d[0])
                nc.sync.dma_start(wd_sb[:, 0], wd_d[0])
                nc.sync.dma_start(axt_sb[:, 1], axt_d[1])
                nc.sync.dma_start(wd_sb[:, 1], wd_d[1])
                nc.sync.dma_start(axt_sb[:, 2], axt_d[2])
                for kt in range(2, 8):
                    nc.sync.dma_start(wd_sb[:, kt], wd_d[kt])
                for i in range(4):
                    nc.sync.dma_start(wd8_sb[:, i], wd8_d[i])
                for b in range(3, 8):
                    nc.sync.dma_start(axt_sb[:, b], axt_d[b])

                # ---------------- Y2 = 8*lrelu(psum/4096): ACT copies PSUM
                # out with scale, DVE applies lrelu (one-PSUM-operand rule)
                for mb in range(2):
                    z = zpool.tile([128, 512], f16, tag="z")
                    nc.scalar.activation(
                        z[:], y2ps[mb][:],
                        mybir.ActivationFunctionType.Copy,
                        bias=0.0, scale=float(SY / SC))
                    nc.vector.scalar_tensor_tensor(
                        Y2[:, mb], z[:], ALPHA, z[:], Alu.mult, Alu.max)

            # ---------------- lhsC8 = fp8(Y2 + 8*W_emb^T) (free-dim slices
            # only; (uv, mb) grouping == DoubleRow k-pair grouping)
            for mb in range(2):
                nc.vector.tensor_tensor(
                    lhsC8[:, :, mb, :],
                    Y2[:, mb],
                    wrows_sb[:, :, mb, :], Alu.add)

            # ---------------- D rows (fp8 DoubleRow) then per-batch row
            # matmuls (fp16), kt-skewed over the first SKEW groups so the
            # PE follows the wd k-tile DMA stream without stalling.
            with tc.tile_pool(name="psD", bufs=1, space="PSUM") as psD, \
                 tc.tile_pool(name="psF", bufs=6, space="PSUM") as psF:

                groups = [(b, nb) for b in range(8) for nb in range(2)]
                fp = {}

                def emit_add(g):
                    b, nb = groups[g]
                    ot = opool.tile([128, 512], f16, tag="ot",
                                    name=f"ot{b}_{nb}")
                    nc.vector.tensor_tensor(
                        ot[:], fp[g][:],
                        drows[:, 512 * nb:512 * (nb + 1)], Alu.add)
                    nc.scalar.dma_start(
                        out_d[b][:, 512 * nb:512 * (nb + 1)], ot[:])

                def emit_mm(g, kt, add=True):
                    b, nb = groups[g]
                    if kt == 0:
                        fp[g] = psF.tile([128, 512], f32, tag="fp",
                                         name=f"fp{b}_{nb}")
                    nc.tensor.matmul(
                        fp[g][:],
                        axt_sb[:, b, 128 * kt:128 * (kt + 1)],
                        wd_sb[:, kt, 512 * nb:512 * (nb + 1)],
                        start=(kt == 0), stop=(kt == 7))
                    if kt == 7 and add:
                        emit_add(g)

                # b0/b1 groups kt-skewed: the PE follows the wd k-tile
                # stream right after the conv without waiting for all of wd.
                # Groups 0..5 hold the 6 psF banks until their adds flush
                # after drows (Vector queue is in-order; the adds depend on
                # it, and bank reuse by later groups must not precede D on
                # the PE queue).
                SKEW = 5
                for t in range(8 + SKEW - 1):
                    for g in range(SKEW):
                        kt = t - g
                        if 0 <= kt < 8:
                            emit_mm(g, kt, add=False)
                for g in range(SKEW, 6):
                    for kt in range(8):
                        emit_mm(g, kt, add=False)

                # D rows (fp8 DoubleRow); wd8 lands about now and the D
                # matmuls fill the axt3/axt4 DMA wait.
                dps = psD.tile([128, 1024], f32, tag="dps")
                for nb in range(2):
                    o = dps[:, 512 * nb:512 * (nb + 1)]
                    nc.tensor.matmul(
                        o, ones_sb[:], bd_sb[:, 512 * nb:512 * (nb + 1)],
                        start=True, stop=False)
                    for tp in range(4):
                        nc.tensor.matmul(
                            o, lhsC8[:, tp, :, :], wd8_sb[:, tp, :, nb, :],
                            start=False, stop=(tp == 3), perf_mode=DR)
                nc.vector.tensor_scalar(
                    drows[:], dps[:], float(1.0 / SD), None, Alu.mult)
                for g in range(6):
                    emit_add(g)

                for g in range(6, 16):
                    for kt in range(8):
                        emit_mm(g, kt)

    nc.compile()
    return nc


def get_nc():
    if "nc" not in _CACHE:
        _CACHE["nc"] = _build()
    return _CACHE["nc"]


# ---------------------------------------------------------------- entry points

def run(inputs, trace=False, **kwargs):
    from concourse.bass_utils import run_bass_kernel_spmd
    nc = get_nc()
    in_maps = host_prep(inputs)
    res = run_bass_kernel_spmd(nc, in_maps, list(range(NC)), trace=trace, **kwargs)
    out = np.empty((B, N0, D0), np.float32)
    for k in range(NC):
        out[:, 128 * k:128 * (k + 1), :] = res.results[k]["out"].astype(np.float32)
    return out, res


def kernel(**inputs):
    out, _ = run(inputs)
    return out


# revision 29
# speedup vs baseline: 1.0246x; 1.0246x over previous
"""Trainium2 Bass kernel for nn_DeepPatchEncoder.

Math: the reference collapses to
    out[b] = A_X[b] @ W_dense + D_const
    D_const = (A_P + W_emb) @ W_dense + b_dense
where A_X[b] is the coarse-patchify permutation of X[b] and A_P is a
permutation of the conv-branch output (conv3x3 s2 on W_emb viewed as a
[32,32,1024] image, then BN + LeakyReLU).

Sharding (zero cross-core communication):
  Core k computes output ROWS n0 in [128k, 128(k+1)) for ALL 8 batches.
  Those rows of D_const need exactly conv output channels [512k, 512k+512),
  which core k computes itself.

Precision strategy: the conv branch and D_const are SMALL additive terms
(|pos2| ~ 0.05 vs |X-patch| ~ 1), so they run in fp8e4m3 with
perf_mode=DoubleRow (2 fp8 weights per PE cell, K=256 per matmul) --
~2x tensor throughput and half the DMA bytes for conv weights.  The
batched main matmul A_X @ W_dense stays fp16 (fp8 would blow the 2e-2
error budget).  Scales (all powers of 2, folded on host):
    pe8 = 64*pe, cw8 = 64*(conv_w*bn_scale)  -> conv PSUM = 4096*y
    Y2  = 8*lrelu(y)   (copy-out scale 1/512; lrelu commutes with +scale)
    wrows = 8*W_emb^T, lhsC8 = Y2 + wrows = 8*lhsC
    wd8 = 32*W_dense   -> D PSUM = 256*D, drows = PSUM/256
Outputs written fp16 (quantum ~2e-3 << 0.068 abs budget), upcast on host.

Schedule: conv data (pe/cw) rides the sync HWDGE ring FIRST and alone
(the ring saturates ~330 GB/s and the conv consumes it at ~408 GB/s,
so any filler stretches the conv window 1:1); tiny tensors (t2, ones,
wrows, bd) ride the scalar ring in parallel so the first conv matmul
only waits on pe0+cw0.  PE warm-up matmuls on memset data run before
the first data matmul so HAM reaches K=8/8.  Post-conv sync ring:
axt0, axt1, wd0-7, axt2, axt3, wd8, axt4-7.  Main-phase matmuls are
emitted kt-skewed across the first 4 PSUM groups (b0/b1) to follow the
wd stream, then b2/b3, then the D matmuls (their wd8 lands about then;
they fill the axt4 wait), then b4-b7.  drows runs on GpSimd so the
in-order Vector queue (output adds) can't deadlock on it; output DMAs
ride the scalar HWDGE ring.
"""

import numpy as np
import ml_dtypes

B = 8
NC = 8
IMG = 1024
N0 = 1024
D0 = 1024
BN_EPS = 1e-3
ALPHA = 0.3

SA = 64.0     # pe scale
SB = 64.0     # cw scale
SC = SA * SB  # conv psum scale = 4096
SY = 8.0      # Y2 / wrows / lhsC scale
SW8 = 32.0    # wd8 scale
SD = SY * SW8 # D psum scale = 256

_CACHE = {}

F8 = ml_dtypes.float8_e4m3


def _f8(x):
    return np.clip(x, -240.0, 240.0).astype(F8)


# ---------------------------------------------------------------- host prep

def _perms():
    # rho2: lhsC position d0'' = 128*k2 + p2 -> natural d0 = 512u+32a+16v+bh
    #   with k2 = 4u + 2v + mb, a = 8*mb + p2//16, bh = p2 % 16
    i = np.arange(1024)
    k2, p2 = i // 128, i % 128
    u, v, mb = k2 // 4, (k2 // 2) % 2, k2 % 2
    a, bh = 8 * mb + p2 // 16, p2 % 16
    rho = 512 * u + 32 * a + 16 * v + bh
    # sigma2: conv rhs column pos2 = 128*(2u+v) + 32*ocb + j0 -> in-shard
    #   channel c_loc = 128*ocb + 64u + 2j0 + v  (so lhsC slices are plain
    #   free-dim slices of the conv output)
    p = np.arange(512)
    uv, ocb, j0 = p // 128, (p // 32) % 4, p % 32
    uu, vv = uv // 2, uv % 2
    sigma = 128 * ocb + 64 * uu + 2 * j0 + vv
    return rho, sigma


def host_prep(inputs):
    f16 = np.float16
    X = np.asarray(inputs["X"], np.float32).reshape(B, IMG, IMG)
    W_emb = np.asarray(inputs["W_emb"], np.float32)
    conv_w = np.asarray(inputs["conv_w"], np.float32)
    conv_b = np.asarray(inputs["conv_b"], np.float32)
    g = np.asarray(inputs["bn_gamma"], np.float32)
    be = np.asarray(inputs["bn_beta"], np.float32)
    mu = np.asarray(inputs["bn_mean"], np.float32)
    var = np.asarray(inputs["bn_var"], np.float32)
    W_dense = np.asarray(inputs["W_dense"], np.float32)
    b_dense = np.asarray(inputs["b_dense"], np.float32)

    rho, sigma = _perms()

    s_all = g / np.sqrt(var + BN_EPS)
    t_all = (conv_b - mu) * s_all + be

    # pe lhsT blocks, per (dd, tp=kt//2, i=kt%2, mb):
    # pe[dd, p, tp, i, mb, 16a'+oj] = peT[128*(2tp+i)+p, di+2*(8mb+a'), dj+2oj]
    peT = np.zeros((N0, 33, 33), np.float32)
    peT[:, :32, :32] = W_emb.reshape(N0, 32, 32)
    peb = np.empty((9, N0, 256), np.float32)
    for dd in range(9):
        di, dj = dd // 3, dd % 3
        blk = peT[:, di:di + 31:2, dj:dj + 31:2]       # [1024, 16, 16]
        peb[dd] = blk.reshape(N0, 256)                 # 16a+oj: a = 8mb+a'
    # free dim (a, oj) == (mb, a', oj); channel 1024 -> (kt-major, p)
    peb = peb.reshape(9, 8, 128, 2, 128).transpose(0, 2, 1, 3, 4)
    pe_host = _f8(peb.reshape(9, 128, 4, 2, 2, 128) * SA)

    # W_dense row-permuted by rho, K-tile major [8, 128, 1024] fp16
    wd_perm = W_dense[rho, :]                                    # [1024, 1024]
    wd_host = np.ascontiguousarray(wd_perm.reshape(8, 128, D0)).astype(f16)
    # fp8 copy for the D matmul, DoubleRow pair layout
    # wd8[tp, p, i, nb, n] = SW8 * W_dense[rho[128*(2tp+i)+p], 512*nb+n]
    wd8_host = _f8(
        (wd_perm * SW8).reshape(4, 2, 128, 2, 512).transpose(0, 2, 1, 3, 4)
    )

    bd_host = (b_dense * SD).reshape(1, D0).astype(f16)

    in_maps = []
    for k in range(NC):
        ch = 512 * k + sigma
        # conv weights as DoubleRow matmul RHS, BN scale s folded in:
        # cw[dd, p, tp, i, oc] = SB * s[ch[oc]] * conv_w[dd, 128*(2tp+i)+p, ch[oc]]
        cws = conv_w[:, :, :, ch] * s_all[ch]          # [3,3,1024,512]
        cw = cws.reshape(9, 8, 128, 512).transpose(0, 2, 1, 3)
        cw_host = _f8(cw.reshape(9, 128, 4, 2, 512) * SB)
        # BN shift as a K=1 seed-matmul rhs row, in PSUM scale
        t2 = (t_all[ch] * SC).reshape(1, 512).astype(f16)
        # A_X^T for this core's row strip, rho-permuted, SBUF layout
        # [8b, 128part, 8kt*128j]: axt[b, p, 128*kt+j] = A_X^T[b, 128kt+p, j]
        Xs = X[:, 128 * k:128 * (k + 1), :]        # [8,128,1024]
        axt = Xs.reshape(B, 4, 32, 32, 32).transpose(0, 2, 4, 1, 3).reshape(B, 1024, 128)
        axt = axt[:, rho, :].reshape(B, 8, 128, 128).transpose(0, 2, 1, 3)
        axt = np.ascontiguousarray(axt.reshape(B, 128, 1024)).astype(f16)
        # W_emb row block transposed into lhsC layout, pre-scaled by SY:
        # wrows[p, 128*k2 + j] = SY * W_emb[128k + j, rho[128*k2 + p]]
        wr = W_emb[128 * k:128 * (k + 1), :][:, rho]        # [128j, 1024d0']
        wrows = np.ascontiguousarray(
            (wr.T * SY).reshape(8, 128, 128).transpose(1, 0, 2).reshape(128, 4, 2, 128)
        ).astype(f16)
        in_maps.append({
            "cw": cw_host, "pe": pe_host, "t2": t2, "axt": axt,
            "wrows": wrows, "wd": wd_host, "wd8": wd8_host, "bd": bd_host,
            "ones": np.ones((1, 128), np.float16),
        })
    return in_maps


# ---------------------------------------------------------------- device code

def _build():
    import concourse.tile as tile
    import concourse.mybir as mybir
    from concourse import bacc

    f32 = mybir.dt.float32
    f16 = mybir.dt.float16
    f8 = mybir.dt.float8e4
    Alu = mybir.AluOpType
    DR = mybir.MatmulPerfMode.DoubleRow

    nc = bacc.Bacc("TRN2", target_bir_lowering=False, debug=False)

    cw_d = nc.dram_tensor("cw", [9, 128, 4, 2, 512], f8, kind="ExternalInput").ap()
    pe_d = nc.dram_tensor("pe", [9, 128, 4, 2, 2, 128], f8, kind="ExternalInput").ap()
    t2_d = nc.dram_tensor("t2", [1, 512], f16, kind="ExternalInput").ap()
    axt_d = nc.dram_tensor("axt", [8, 128, 1024], f16, kind="ExternalInput").ap()
    wrows_d = nc.dram_tensor("wrows", [128, 4, 2, 128], f16, kind="ExternalInput").ap()
    wd_d = nc.dram_tensor("wd", [8, 128, 1024], f16, kind="ExternalInput").ap()
    wd8_d = nc.dram_tensor("wd8", [4, 128, 2, 2, 512], f8, kind="ExternalInput").ap()
    bd_d = nc.dram_tensor("bd", [1, 1024], f16, kind="ExternalInput").ap()
    ones_d = nc.dram_tensor("ones", [1, 128], f16, kind="ExternalInput").ap()
    out_d = nc.dram_tensor("out", [8, 128, 1024], f16, kind="ExternalOutput").ap()

    with tile.TileContext(nc) as tc:
        with (
            tc.tile_pool(name="kpool", bufs=1) as kpool,
            tc.tile_pool(name="zpool", bufs=2) as zpool,
            tc.tile_pool(name="opool", bufs=8) as opool,
        ):
            # persistent SBUF tensors
            pe_sb = kpool.tile([128, 9, 4, 2, 2, 128], f8, tag="pe")
            cw_sb = kpool.tile([128, 9, 4, 2, 512], f8, tag="cw")
            t2_sb = kpool.tile([1, 512], f16, tag="t2")
            Y2 = kpool.tile([128, 2, 512], f16, tag="Y2")   # 8*lrelu(BN(conv))
            lhsC8 = kpool.tile([128, 4, 2, 128], f8, tag="lhsC")
            drows = kpool.tile([128, 1024], f16, tag="drows")
            wrows_sb = kpool.tile([128, 4, 2, 128], f16, tag="wrows")
            bd_sb = kpool.tile([1, 1024], f16, tag="bd")
            ones_sb = kpool.tile([1, 128], f16, tag="ones")
            wd_sb = kpool.tile([128, 8, 1024], f16, tag="wd")
            wd8_sb = kpool.tile([128, 4, 2, 2, 512], f8, tag="wd8")
            axt_sb = kpool.tile([128, 8, 1024], f16, tag="axt")

            # ---------------- conv with BN folded into the PE:
            #   scale s folded into cw on host; shift t is a K=1 seed
            #   matmul (ones^T @ t2) accumulated first into each PSUM bank.
            # DoubleRow fp8: lhsT = pe [128ch, 2kt, 128m], rhs = cw
            # [128ch, 2kt, 512oc]; each MM contracts K=256.
            with tc.tile_pool(name="psA", bufs=1, space="PSUM") as psA, \
                 tc.tile_pool(name="psW", bufs=1, space="PSUM") as psW:
                y2ps = [psA.tile([128, 512], f32, tag=f"y2{mb}", name=f"y2ps{mb}")
                        for mb in range(2)]
                # t2/ones lead the sync ring (tiny) so the seeds run just
                # before the first conv MM; wrows/bd ride the scalar ring
                nc.sync.dma_start(t2_sb[:], t2_d[:])
                nc.sync.dma_start(ones_sb[:], ones_d[:])
                nc.scalar.dma_start(wrows_sb[:], wrows_d[:])
                nc.scalar.dma_start(bd_sb[:], bd_d[:])
                # PE warm-up on memset data (no DMA deps; gpsimd memsets run
                # earliest): HAM hits K=8/8 before the first data matmul
                wps = psW.tile([128, 512], f32, tag="wps")
                wsrc = kpool.tile([1, 128], f16, tag="wsrc")
                wpad = kpool.tile([1, 512], f16, tag="wpad")
                nc.gpsimd.memset(wsrc[:], 1.0)
                nc.gpsimd.memset(wpad[:], 0.0)
                for w in range(6):
                    nc.tensor.matmul(
                        wps[:], wsrc[:], wpad[:],
                        start=(w == 0), stop=(w == 5))
                for mb in range(2):
                    nc.tensor.matmul(
                        y2ps[mb][:], ones_sb[:], t2_sb[:],
                        start=True, stop=False)
                # conv data on the sync ring in consumption order; one DMA
                # per tensor per round (each dma_start costs ~0.6us of
                # engine issue time, so finer splits lose to issue rate)
                for dd in range(9):
                    if dd == 0:
                        # halves, so the first MMs start one transfer earlier
                        nc.sync.dma_start(pe_sb[:, 0, :2], pe_d[0][:, :2])
                        nc.sync.dma_start(cw_sb[:, 0, :2], cw_d[0][:, :2])
                        nc.sync.dma_start(pe_sb[:, 0, 2:], pe_d[0][:, 2:])
                        nc.sync.dma_start(cw_sb[:, 0, 2:], cw_d[0][:, 2:])
                    else:
                        nc.sync.dma_start(pe_sb[:, dd], pe_d[dd])
                        nc.sync.dma_start(cw_sb[:, dd], cw_d[dd])
                    for mb in range(2):
                        for tp in range(4):
                            nc.tensor.matmul(
                                y2ps[mb][:],
                                pe_sb[:, dd, tp, :, mb, :],
                                cw_sb[:, dd, tp, :, :],
                                start=False,
                                stop=(dd == 8 and tp == 3 and mb == 1),
                                perf_mode=DR)
                # dense-path loads, in consumption order
                nc.sync.dma_start(axt_sb[:, 0], axt_d[0])
                nc.sync.dma_start(wd_sb[:, 0], wd_d[0])
                nc.sync.dma_start(axt_sb[:, 1], axt_d[1])
                nc.sync.dma_start(wd_sb[:, 1], wd_d[1])
                nc.sync.dma_start(axt_sb[:, 2], axt_d[2])
                for kt in range(2, 8):
                    nc.sync.dma_start(wd_sb[:, kt], wd_d[kt])
                for i in range(4):
                    nc.sync.dma_start(wd8_sb[:, i], wd8_d[i])
                for b in range(3, 8):
                    nc.sync.dma_start(axt_sb[:, b], axt_d[b])

                # ---------------- Y2 = 8*lrelu(psum/4096): ACT copies PSUM
                # out with scale, DVE applies lrelu (one-PSUM-operand rule)
                for mb in range(2):
                    z = zpool.tile([128, 512], f16, tag="z")
                    nc.scalar.activation(
                        z[:], y2ps[mb][:],
                        mybir.ActivationFunctionType.Copy,
                        bias=0.0, scale=float(SY / SC))
                    nc.vector.scalar_tensor_tensor(
                        Y2[:, mb], z[:], ALPHA, z[:], Alu.mult, Alu.max)

            # ---------------- lhsC8 = fp8(Y2 + 8*W_emb^T) (free-dim slices
            # only; (uv, mb) grouping == DoubleRow k-pair grouping)
            for mb in range(2):
                nc.vector.tensor_tensor(
                    lhsC8[:, :, mb, :],
                    Y2[:, mb],
                    wrows_sb[:, :, mb, :], Alu.add)

            # ---------------- D rows (fp8 DoubleRow) then per-batch row
            # matmuls (fp16), kt-skewed over the first SKEW groups so the
            # PE follows the wd k-tile DMA stream without stalling.
            with tc.tile_pool(name="psD", bufs=1, space="PSUM") as psD, \
                 tc.tile_pool(name="psF", bufs=6, space="PSUM") as psF:

                groups = [(b, nb) for b in range(8) for nb in range(2)]
                fp = {}

                def emit_add(g):
                    b, nb = groups[g]
                    ot = opool.tile([128, 512], f16, tag="ot",
                                    name=f"ot{b}_{nb}")
                    nc.vector.tensor_tensor(
                        ot[:], fp[g][:],
                        drows[:, 512 * nb:512 * (nb + 1)], Alu.add)
                    nc.scalar.dma_start(
                        out_d[b][:, 512 * nb:512 * (nb + 1)], ot[:])

                def emit_mm(g, kt, add=True):
                    b, nb = groups[g]
                    if kt == 0:
                        fp[g] = psF.tile([128, 512], f32, tag="fp",
                                         name=f"fp{b}_{nb}")
                    nc.tensor.matmul(
                        fp[g][:],
                        axt_sb[:, b, 128 * kt:128 * (kt + 1)],
                        wd_sb[:, kt, 512 * nb:512 * (nb + 1)],
                        start=(kt == 0), stop=(kt == 7))
                    if kt == 7 and add:
                        emit_add(g)

                # b0/b1 groups kt-skewed: the PE follows the wd k-tile
                # stream right after the conv without waiting for all of wd.
                # Groups 0..5 hold the 6 psF banks until their adds flush
                # after drows (Vector queue is in-order; the adds depend on
                # it, and bank reuse by later groups must not precede D on
                # the PE queue).
                SKEW = 5
                for t in range(8 + SKEW - 1):
                    for g in range(SKEW):
                        kt = t - g
                        if 0 <= kt < 8:
                            emit_mm(g, kt, add=False)
                for g in range(SKEW, 6):
                    for kt in range(8):
                        emit_mm(g, kt, add=False)

                # D rows (fp8 DoubleRow); wd8 lands about now and the D
                # matmuls fill the axt3/axt4 DMA wait.
                dps = psD.tile([128, 1024], f32, tag="dps")
                for nb in range(2):
                    o = dps[:, 512 * nb:512 * (nb + 1)]
                    nc.tensor.matmul(
                        o, ones_sb[:], bd_sb[:, 512 * nb:512 * (nb + 1)],
                        start=True, stop=False)
                    for tp in range(4):
                        nc.tensor.matmul(
                            o, lhsC8[:, tp, :, :], wd8_sb[:, tp, :, nb, :],
                            start=False, stop=(tp == 3), perf_mode=DR)
                nc.vector.tensor_scalar(
                    drows[:], dps[:], float(1.0 / SD), None, Alu.mult)
                for g in range(6):
                    emit_add(g)

                for g in range(6, 16):
                    for kt in range(8):
                        emit_mm(g, kt)

    nc.compile()
    return nc


def get_nc():
    if "nc" not in _CACHE:
        _CACHE["nc"] = _build()
    return _CACHE["nc"]


# ---------------------------------------------------------------- entry points

def run(inputs, trace=False, **kwargs):
    from concourse.bass_utils import run_bass_kernel_spmd
    nc = get_nc()
    in_maps = host_prep(inputs)
    res = run_bass_kernel_spmd(nc, in_maps, list(range(NC)), trace=trace, **kwargs)
    out = np.empty((B, N0, D0), np.float32)
    for k in range(NC):
        out[:, 128 * k:128 * (k + 1), :] = res.results[k]["out"].astype(np.float32)
    return out, res


def kernel(**inputs):
    out, _ = run(inputs)
    return out


# revision 30
# speedup vs baseline: 1.0644x; 1.0388x over previous
"""Trainium2 Bass kernel for nn_DeepPatchEncoder.

Math: the reference collapses to
    out[b] = A_X[b] @ W_dense + D_const
    D_const = (A_P + W_emb) @ W_dense + b_dense
where A_X[b] is the coarse-patchify permutation of X[b] and A_P is a
permutation of the conv-branch output (conv3x3 s2 on W_emb viewed as a
[32,32,1024] image, then BN + LeakyReLU).

Sharding (zero cross-core communication):
  Core k computes output ROWS n0 in [128k, 128(k+1)) for ALL 8 batches.
  Those rows of D_const need exactly conv output channels [512k, 512k+512),
  which core k computes itself.

Precision strategy: the conv branch and D_const are SMALL additive terms
(|pos2| ~ 0.05 vs |X-patch| ~ 1), so they run in fp8e4m3 with
perf_mode=DoubleRow (2 fp8 weights per PE cell, K=256 per matmul) --
~2x tensor throughput and half the DMA bytes for conv weights.  The
batched main matmul A_X @ W_dense stays fp16 (fp8 would blow the 2e-2
error budget).  Scales (all powers of 2, folded on host):
    pe8 = 64*pe, cw8 = 64*(conv_w*bn_scale)  -> conv PSUM = 4096*y
    Y2  = 8*lrelu(y)   (copy-out scale 1/512; lrelu commutes with +scale)
    wrows = 8*W_emb^T, lhsC8 = Y2 + wrows = 8*lhsC
    wd8 = 32*W_dense   -> D PSUM = 256*D, drows = PSUM/256
Outputs written fp16 (quantum ~2e-3 << 0.068 abs budget), upcast on host.

Schedule: conv data (pe/cw) rides the sync HWDGE ring FIRST and alone
(the ring saturates ~330 GB/s and the conv consumes it at ~408 GB/s,
so any filler stretches the conv window 1:1); tiny tensors (t2, ones,
wrows, bd) ride the scalar ring in parallel so the first conv matmul
only waits on pe0+cw0.  PE warm-up matmuls on memset data run before
the first data matmul so HAM reaches K=8/8.  Post-conv sync ring:
axt0, axt1, wd0-7, axt2, axt3, wd8, axt4-7.  Main-phase matmuls are
emitted kt-skewed across the first 4 PSUM groups (b0/b1) to follow the
wd stream, then b2/b3, then the D matmuls (their wd8 lands about then;
they fill the axt4 wait), then b4-b7.  drows runs on GpSimd so the
in-order Vector queue (output adds) can't deadlock on it; output DMAs
ride the scalar HWDGE ring.
"""

import numpy as np
import ml_dtypes

B = 8
NC = 8
IMG = 1024
N0 = 1024
D0 = 1024
BN_EPS = 1e-3
ALPHA = 0.3

SA = 64.0     # pe scale
SB = 64.0     # cw scale
SC = SA * SB  # conv psum scale = 4096
SY = 8.0      # Y2 / wrows / lhsC scale
SW8 = 32.0    # wd8 scale
SD = SY * SW8 # D psum scale = 256

_CACHE = {}

F8 = ml_dtypes.float8_e4m3


def _f8(x):
    return np.clip(x, -240.0, 240.0).astype(F8)


# ---------------------------------------------------------------- host prep

def _perms():
    # rho2: lhsC position d0'' = 128*k2 + p2 -> natural d0 = 512u+32a+16v+bh
    #   with k2 = 4u + 2v + mb, a = 8*mb + p2//16, bh = p2 % 16
    i = np.arange(1024)
    k2, p2 = i // 128, i % 128
    u, v, mb = k2 // 4, (k2 // 2) % 2, k2 % 2
    a, bh = 8 * mb + p2 // 16, p2 % 16
    rho = 512 * u + 32 * a + 16 * v + bh
    # sigma2: conv rhs column pos2 = 128*(2u+v) + 32*ocb + j0 -> in-shard
    #   channel c_loc = 128*ocb + 64u + 2j0 + v  (so lhsC slices are plain
    #   free-dim slices of the conv output)
    p = np.arange(512)
    uv, ocb, j0 = p // 128, (p // 32) % 4, p % 32
    uu, vv = uv // 2, uv % 2
    sigma = 128 * ocb + 64 * uu + 2 * j0 + vv
    return rho, sigma


def host_prep(inputs):
    f16 = np.float16
    X = np.asarray(inputs["X"], np.float32).reshape(B, IMG, IMG)
    W_emb = np.asarray(inputs["W_emb"], np.float32)
    conv_w = np.asarray(inputs["conv_w"], np.float32)
    conv_b = np.asarray(inputs["conv_b"], np.float32)
    g = np.asarray(inputs["bn_gamma"], np.float32)
    be = np.asarray(inputs["bn_beta"], np.float32)
    mu = np.asarray(inputs["bn_mean"], np.float32)
    var = np.asarray(inputs["bn_var"], np.float32)
    W_dense = np.asarray(inputs["W_dense"], np.float32)
    b_dense = np.asarray(inputs["b_dense"], np.float32)

    rho, sigma = _perms()

    s_all = g / np.sqrt(var + BN_EPS)
    t_all = (conv_b - mu) * s_all + be

    # pe lhsT blocks, per (dd, tp=kt//2, i=kt%2, mb):
    # pe[dd, p, tp, i, mb, 16a'+oj] = peT[128*(2tp+i)+p, di+2*(8mb+a'), dj+2oj]
    peT = np.zeros((N0, 33, 33), np.float32)
    peT[:, :32, :32] = W_emb.reshape(N0, 32, 32)
    peb = np.empty((9, N0, 256), np.float32)
    for dd in range(9):
        di, dj = dd // 3, dd % 3
        blk = peT[:, di:di + 31:2, dj:dj + 31:2]       # [1024, 16, 16]
        peb[dd] = blk.reshape(N0, 256)                 # 16a+oj: a = 8mb+a'
    # free dim (a, oj) == (mb, a', oj); channel 1024 -> (kt-major, p)
    peb = peb.reshape(9, 8, 128, 2, 128).transpose(0, 2, 1, 3, 4)
    pe_host = _f8(peb.reshape(9, 128, 4, 2, 2, 128) * SA)

    # W_dense row-permuted by rho, K-tile major [8, 128, 1024] fp16
    wd_perm = W_dense[rho, :]                                    # [1024, 1024]
    wd_host = np.ascontiguousarray(wd_perm.reshape(8, 128, D0)).astype(f16)
    # fp8 copy for the D matmul, DoubleRow pair layout
    # wd8[tp, p, i, nb, n] = SW8 * W_dense[rho[128*(2tp+i)+p], 512*nb+n]
    wd8_host = _f8(
        (wd_perm * SW8).reshape(4, 2, 128, 2, 512).transpose(0, 2, 1, 3, 4)
    )

    bd_host = (b_dense * SD).reshape(1, D0).astype(f16)

    in_maps = []
    for k in range(NC):
        ch = 512 * k + sigma
        # conv weights as DoubleRow matmul RHS, BN scale s folded in:
        # cw[dd, p, tp, i, oc] = SB * s[ch[oc]] * conv_w[dd, 128*(2tp+i)+p, ch[oc]]
        cws = conv_w[:, :, :, ch] * s_all[ch]          # [3,3,1024,512]
        cw = cws.reshape(9, 8, 128, 512).transpose(0, 2, 1, 3)
        cw_host = _f8(cw.reshape(9, 128, 4, 2, 512) * SB)
        # BN shift as a K=1 seed-matmul rhs row, in PSUM scale
        t2 = (t_all[ch] * SC).reshape(1, 512).astype(f16)
        # A_X^T for this core's row strip, rho-permuted, SBUF layout
        # [8b, 128part, 8kt*128j]: axt[b, p, 128*kt+j] = A_X^T[b, 128kt+p, j]
        Xs = X[:, 128 * k:128 * (k + 1), :]        # [8,128,1024]
        axt = Xs.reshape(B, 4, 32, 32, 32).transpose(0, 2, 4, 1, 3).reshape(B, 1024, 128)
        axt = axt[:, rho, :].reshape(B, 8, 128, 128).transpose(0, 2, 1, 3)
        axt = np.ascontiguousarray(axt.reshape(B, 128, 1024)).astype(f16)
        # W_emb row block transposed into lhsC layout, pre-scaled by SY:
        # wrows[p, 128*k2 + j] = SY * W_emb[128k + j, rho[128*k2 + p]]
        wr = W_emb[128 * k:128 * (k + 1), :][:, rho]        # [128j, 1024d0']
        wrows = np.ascontiguousarray(
            (wr.T * SY).reshape(8, 128, 128).transpose(1, 0, 2).reshape(128, 4, 2, 128)
        ).astype(f16)
        in_maps.append({
            "cw": cw_host, "pe": pe_host, "t2": t2, "axt": axt,
            "wrows": wrows, "wd": wd_host, "wd8": wd8_host, "bd": bd_host,
            "ones": np.ones((1, 128), np.float16),
        })
    return in_maps


# ---------------------------------------------------------------- device code

def _build():
    import concourse.tile as tile
    import concourse.mybir as mybir
    from concourse import bacc

    f32 = mybir.dt.float32
    f16 = mybir.dt.float16
    f8 = mybir.dt.float8e4
    Alu = mybir.AluOpType
    DR = mybir.MatmulPerfMode.DoubleRow

    nc = bacc.Bacc("TRN2", target_bir_lowering=False, debug=False)

    cw_d = nc.dram_tensor("cw", [9, 128, 4, 2, 512], f8, kind="ExternalInput").ap()
    pe_d = nc.dram_tensor("pe", [9, 128, 4, 2, 2, 128], f8, kind="ExternalInput").ap()
    t2_d = nc.dram_tensor("t2", [1, 512], f16, kind="ExternalInput").ap()
    axt_d = nc.dram_tensor("axt", [8, 128, 1024], f16, kind="ExternalInput").ap()
    wrows_d = nc.dram_tensor("wrows", [128, 4, 2, 128], f16, kind="ExternalInput").ap()
    wd_d = nc.dram_tensor("wd", [8, 128, 1024], f16, kind="ExternalInput").ap()
    wd8_d = nc.dram_tensor("wd8", [4, 128, 2, 2, 512], f8, kind="ExternalInput").ap()
    bd_d = nc.dram_tensor("bd", [1, 1024], f16, kind="ExternalInput").ap()
    ones_d = nc.dram_tensor("ones", [1, 128], f16, kind="ExternalInput").ap()
    out_d = nc.dram_tensor("out", [8, 128, 1024], f16, kind="ExternalOutput").ap()

    with tile.TileContext(nc) as tc:
        with (
            tc.tile_pool(name="kpool", bufs=1) as kpool,
            tc.tile_pool(name="zpool", bufs=2) as zpool,
            tc.tile_pool(name="opool", bufs=8) as opool,
        ):
            # persistent SBUF tensors
            pe_sb = kpool.tile([128, 9, 4, 2, 2, 128], f8, tag="pe")
            cw_sb = kpool.tile([128, 9, 4, 2, 512], f8, tag="cw")
            t2_sb = kpool.tile([1, 512], f16, tag="t2")
            Y2 = kpool.tile([128, 2, 512], f16, tag="Y2")   # 8*lrelu(BN(conv))
            lhsC8 = kpool.tile([128, 4, 2, 128], f8, tag="lhsC")
            drows = kpool.tile([128, 1024], f16, tag="drows")
            wrows_sb = kpool.tile([128, 4, 2, 128], f16, tag="wrows")
            bd_sb = kpool.tile([1, 1024], f16, tag="bd")
            ones_sb = kpool.tile([1, 128], f16, tag="ones")
            wd_sb = kpool.tile([128, 8, 1024], f16, tag="wd")
            wd8_sb = kpool.tile([128, 4, 2, 2, 512], f8, tag="wd8")
            axt_sb = kpool.tile([128, 8, 1024], f16, tag="axt")

            # ---------------- conv with BN folded into the PE:
            #   scale s folded into cw on host; shift t is a K=1 seed
            #   matmul (ones^T @ t2) accumulated first into each PSUM bank.
            # DoubleRow fp8: lhsT = pe [128ch, 2kt, 128m], rhs = cw
            # [128ch, 2kt, 512oc]; each MM contracts K=256.
            with tc.tile_pool(name="psA", bufs=1, space="PSUM") as psA, \
                 tc.tile_pool(name="psW", bufs=1, space="PSUM") as psW:
                y2ps = [psA.tile([128, 512], f32, tag=f"y2{mb}", name=f"y2ps{mb}")
                        for mb in range(2)]
                # t2/ones lead the sync ring (tiny) so the seeds run just
                # before the first conv MM; wrows/bd ride the scalar ring
                nc.sync.dma_start(t2_sb[:], t2_d[:])
                nc.sync.dma_start(ones_sb[:], ones_d[:])
                nc.scalar.dma_start(wrows_sb[:], wrows_d[:])
                nc.scalar.dma_start(bd_sb[:], bd_d[:])
                # PE warm-up on memset data (no DMA deps; gpsimd memsets run
                # earliest): HAM hits K=8/8 before the first data matmul
                wps = psW.tile([128, 512], f32, tag="wps")
                wsrc = kpool.tile([1, 128], f16, tag="wsrc")
                wpad = kpool.tile([1, 512], f16, tag="wpad")
                nc.gpsimd.memset(wsrc[:], 1.0)
                nc.gpsimd.memset(wpad[:], 0.0)
                for w in range(6):
                    nc.tensor.matmul(
                        wps[:], wsrc[:], wpad[:],
                        start=(w == 0), stop=(w == 5))
                for mb in range(2):
                    nc.tensor.matmul(
                        y2ps[mb][:], ones_sb[:], t2_sb[:],
                        start=True, stop=False)
                # conv data on the sync ring in consumption order; one DMA
                # per tensor per round (each dma_start costs ~0.6us of
                # engine issue time, so finer splits lose to issue rate)
                for dd in range(9):
                    if dd == 0:
                        # halves, so the first MMs start one transfer earlier
                        nc.sync.dma_start(pe_sb[:, 0, :2], pe_d[0][:, :2])
                        nc.sync.dma_start(cw_sb[:, 0, :2], cw_d[0][:, :2])
                        nc.sync.dma_start(pe_sb[:, 0, 2:], pe_d[0][:, 2:])
                        nc.sync.dma_start(cw_sb[:, 0, 2:], cw_d[0][:, 2:])
                    else:
                        nc.sync.dma_start(pe_sb[:, dd], pe_d[dd])
                        nc.sync.dma_start(cw_sb[:, dd], cw_d[dd])
                    for mb in range(2):
                        for tp in range(4):
                            nc.tensor.matmul(
                                y2ps[mb][:],
                                pe_sb[:, dd, tp, :, mb, :],
                                cw_sb[:, dd, tp, :, :],
                                start=False,
                                stop=(dd == 8 and tp == 3 and mb == 1),
                                perf_mode=DR)
                # dense-path loads, in consumption order; wd8 quarters are
                # interleaved into the wd stream so the D matmuls never
                # wait long enough to cross the 3.4us HAM re-throttle
                # window (a half-clock cliff for the rest of the phase)
                nc.sync.dma_start(axt_sb[:, 0], axt_d[0])
                nc.sync.dma_start(wd_sb[:, 0], wd_d[0])
                nc.sync.dma_start(axt_sb[:, 1], axt_d[1])
                nc.sync.dma_start(wd_sb[:, 1], wd_d[1])
                nc.sync.dma_start(wd_sb[:, 2], wd_d[2])
                for i in range(4):
                    nc.sync.dma_start(wd8_sb[:, i], wd8_d[i])
                    nc.sync.dma_start(wd_sb[:, 3 + i], wd_d[3 + i])
                nc.sync.dma_start(wd_sb[:, 7], wd_d[7])
                for b in range(2, 8):
                    nc.sync.dma_start(axt_sb[:, b], axt_d[b])

                # ---------------- Y2 = 8*lrelu(psum/4096): ACT copies PSUM
                # out with scale, DVE applies lrelu (one-PSUM-operand rule)
                for mb in range(2):
                    z = zpool.tile([128, 512], f16, tag="z")
                    nc.scalar.activation(
                        z[:], y2ps[mb][:],
                        mybir.ActivationFunctionType.Copy,
                        bias=0.0, scale=float(SY / SC))
                    nc.vector.scalar_tensor_tensor(
                        Y2[:, mb], z[:], ALPHA, z[:], Alu.mult, Alu.max)

            # ---------------- lhsC8 = fp8(Y2 + 8*W_emb^T) (free-dim slices
            # only; (uv, mb) grouping == DoubleRow k-pair grouping)
            for mb in range(2):
                nc.vector.tensor_tensor(
                    lhsC8[:, :, mb, :],
                    Y2[:, mb],
                    wrows_sb[:, :, mb, :], Alu.add)

            # ---------------- D rows (fp8 DoubleRow) then per-batch row
            # matmuls (fp16), kt-skewed over the first SKEW groups so the
            # PE follows the wd k-tile DMA stream without stalling.
            with tc.tile_pool(name="psD", bufs=1, space="PSUM") as psD, \
                 tc.tile_pool(name="psF", bufs=6, space="PSUM") as psF:

                groups = [(b, nb) for b in range(8) for nb in range(2)]
                fp = {}

                def emit_add(g):
                    b, nb = groups[g]
                    ot = opool.tile([128, 512], f16, tag="ot",
                                    name=f"ot{b}_{nb}")
                    nc.vector.tensor_tensor(
                        ot[:], fp[g][:],
                        drows[:, 512 * nb:512 * (nb + 1)], Alu.add)
                    nc.scalar.dma_start(
                        out_d[b][:, 512 * nb:512 * (nb + 1)], ot[:])

                def emit_mm(g, kt, add=True):
                    b, nb = groups[g]
                    if kt == 0:
                        fp[g] = psF.tile([128, 512], f32, tag="fp",
                                         name=f"fp{b}_{nb}")
                    nc.tensor.matmul(
                        fp[g][:],
                        axt_sb[:, b, 128 * kt:128 * (kt + 1)],
                        wd_sb[:, kt, 512 * nb:512 * (nb + 1)],
                        start=(kt == 0), stop=(kt == 7))
                    if kt == 7 and add:
                        emit_add(g)

                # b0/b1 groups kt-skewed: the PE follows the wd k-tile
                # stream right after the conv without waiting for all of wd.
                # Groups 0..5 hold the 6 psF banks until their adds flush
                # after drows (Vector queue is in-order; the adds depend on
                # it, and bank reuse by later groups must not precede D on
                # the PE queue).
                SKEW = 5
                for t in range(8 + SKEW - 1):
                    for g in range(SKEW):
                        kt = t - g
                        if 0 <= kt < 8:
                            emit_mm(g, kt, add=False)
                for g in range(SKEW, 6):
                    for kt in range(8):
                        emit_mm(g, kt, add=False)

                # D rows (fp8 DoubleRow); wd8 lands about now and the D
                # matmuls fill the axt3/axt4 DMA wait.
                dps = psD.tile([128, 1024], f32, tag="dps")
                for nb in range(2):
                    o = dps[:, 512 * nb:512 * (nb + 1)]
                    nc.tensor.matmul(
                        o, ones_sb[:], bd_sb[:, 512 * nb:512 * (nb + 1)],
                        start=True, stop=False)
                    for tp in range(4):
                        nc.tensor.matmul(
                            o, lhsC8[:, tp, :, :], wd8_sb[:, tp, :, nb, :],
                            start=False, stop=(tp == 3), perf_mode=DR)
                nc.vector.tensor_scalar(
                    drows[:], dps[:], float(1.0 / SD), None, Alu.mult)
                for g in range(6):
                    emit_add(g)

                for g in range(6, 16):
                    for kt in range(8):
                        emit_mm(g, kt)

    nc.compile()
    return nc


def get_nc():
    if "nc" not in _CACHE:
        _CACHE["nc"] = _build()
    return _CACHE["nc"]


# ---------------------------------------------------------------- entry points

def run(inputs, trace=False, **kwargs):
    from concourse.bass_utils import run_bass_kernel_spmd
    nc = get_nc()
    in_maps = host_prep(inputs)
    res = run_bass_kernel_spmd(nc, in_maps, list(range(NC)), trace=trace, **kwargs)
    out = np.empty((B, N0, D0), np.float32)
    for k in range(NC):
        out[:, 128 * k:128 * (k + 1), :] = res.results[k]["out"].astype(np.float32)
    return out, res


def kernel(**inputs):
    out, _ = run(inputs)
    return out


# revision 31
# speedup vs baseline: 1.1122x; 1.0449x over previous
"""Trainium2 Bass kernel for nn_DeepPatchEncoder.

Math: the reference collapses to
    out[b] = A_X[b] @ W_dense + D_const
    D_const = (A_P + W_emb) @ W_dense + b_dense
where A_X[b] is the coarse-patchify permutation of X[b] and A_P is a
permutation of the conv-branch output (conv3x3 s2 on W_emb viewed as a
[32,32,1024] image, then BN + LeakyReLU).

Sharding (zero cross-core communication):
  Core k computes output ROWS n0 in [128k, 128(k+1)) for ALL 8 batches.
  Those rows of D_const need exactly conv output channels [512k, 512k+512),
  which core k computes itself.

Precision strategy: the conv branch and D_const are SMALL additive terms
(|pos2| ~ 0.05 vs |X-patch| ~ 1), so they run in fp8e4m3 with
perf_mode=DoubleRow (2 fp8 weights per PE cell, K=256 per matmul) --
~2x tensor throughput and half the DMA bytes for conv weights.  The
batched main matmul A_X @ W_dense stays fp16 (fp8 would blow the 2e-2
error budget).  Scales (all powers of 2, folded on host):
    pe8 = 64*pe, cw8 = 64*(conv_w*bn_scale)  -> conv PSUM = 4096*y
    Y2  = 8*lrelu(y)   (copy-out scale 1/512; lrelu commutes with +scale)
    wrows = 8*W_emb^T, lhsC8 = Y2 + wrows = 8*lhsC
    wd8 = 32*W_dense   -> D PSUM = 256*D, drows = PSUM/256
Outputs written fp16 (quantum ~2e-3 << 0.068 abs budget), upcast on host.

Schedule (measured on HW, exec ~70us vs 98.4us baseline):
  - conv data (pe/cw) rides the sync HWDGE ring FIRST and alone, in
    per-round consumption order (ring saturates ~330 GB/s; conv
    consumes ~408 GB/s, so the conv is mildly DMA-bound and any filler
    stretches it 1:1).  One DMA per tensor per round -- each dma_start
    costs ~0.6us of engine issue time, so finer splits lose.
  - t2/ones lead the sync ring (seeds run just before conv); wrows/bd
    ride the otherwise-idle scalar ring.
  - 6 PE warm-up matmuls on memset data (issued from ~8us, before any
    DMA lands) push HAM to K=8/8 around the first data matmul.
  - post-conv sync ring: axt0, wd0, axt1, wd1, wd2, then wd8 quarters
    interleaved with wd3-6, wd7, axt2-7.  The interleave keeps every
    PE wait under the 3.4us HAM re-throttle window (crossing it
    halves the PE clock for ~ the rest of the phase).
  - main-phase matmuls kt-skewed across 5 PSUM groups (b0/b1/b2) so
    the PE follows the wd k-tile stream right out of the conv; then
    (2,1); then the D matmuls (wd8 long since resident); groups 0-5
    hold the 6 psF banks and flush their output adds after drows
    (in-order Vector queue: the adds depend on drows, and psF bank
    reuse by later groups must not precede D on the PE queue); then
    b3-b7 with inline adds.
  - output DMAs ride the scalar HWDGE ring.
"""

import numpy as np
import ml_dtypes

B = 8
NC = 8
IMG = 1024
N0 = 1024
D0 = 1024
BN_EPS = 1e-3
ALPHA = 0.3

SA = 64.0     # pe scale
SB = 64.0     # cw scale
SC = SA * SB  # conv psum scale = 4096
SY = 8.0      # Y2 / wrows / lhsC scale
SW8 = 32.0    # wd8 scale
SD = SY * SW8 # D psum scale = 256

_CACHE = {}

F8 = ml_dtypes.float8_e4m3


def _f8(x):
    return np.clip(x, -240.0, 240.0).astype(F8)


# ---------------------------------------------------------------- host prep

def _perms():
    # rho2: lhsC position d0'' = 128*k2 + p2 -> natural d0 = 512u+32a+16v+bh
    #   with k2 = 4u + 2v + mb, a = 8*mb + p2//16, bh = p2 % 16
    i = np.arange(1024)
    k2, p2 = i // 128, i % 128
    u, v, mb = k2 // 4, (k2 // 2) % 2, k2 % 2
    a, bh = 8 * mb + p2 // 16, p2 % 16
    rho = 512 * u + 32 * a + 16 * v + bh
    # sigma2: conv rhs column pos2 = 128*(2u+v) + 32*ocb + j0 -> in-shard
    #   channel c_loc = 128*ocb + 64u + 2j0 + v  (so lhsC slices are plain
    #   free-dim slices of the conv output)
    p = np.arange(512)
    uv, ocb, j0 = p // 128, (p // 32) % 4, p % 32
    uu, vv = uv // 2, uv % 2
    sigma = 128 * ocb + 64 * uu + 2 * j0 + vv
    return rho, sigma


def host_prep(inputs):
    f16 = np.float16
    X = np.asarray(inputs["X"], np.float32).reshape(B, IMG, IMG)
    W_emb = np.asarray(inputs["W_emb"], np.float32)
    conv_w = np.asarray(inputs["conv_w"], np.float32)
    conv_b = np.asarray(inputs["conv_b"], np.float32)
    g = np.asarray(inputs["bn_gamma"], np.float32)
    be = np.asarray(inputs["bn_beta"], np.float32)
    mu = np.asarray(inputs["bn_mean"], np.float32)
    var = np.asarray(inputs["bn_var"], np.float32)
    W_dense = np.asarray(inputs["W_dense"], np.float32)
    b_dense = np.asarray(inputs["b_dense"], np.float32)

    rho, sigma = _perms()

    s_all = g / np.sqrt(var + BN_EPS)
    t_all = (conv_b - mu) * s_all + be

    # pe lhsT blocks, per (dd, tp=kt//2, i=kt%2, mb):
    # pe[dd, p, tp, i, mb, 16a'+oj] = peT[128*(2tp+i)+p, di+2*(8mb+a'), dj+2oj]
    peT = np.zeros((N0, 33, 33), np.float32)
    peT[:, :32, :32] = W_emb.reshape(N0, 32, 32)
    peb = np.empty((9, N0, 256), np.float32)
    for dd in range(9):
        di, dj = dd // 3, dd % 3
        blk = peT[:, di:di + 31:2, dj:dj + 31:2]       # [1024, 16, 16]
        peb[dd] = blk.reshape(N0, 256)                 # 16a+oj: a = 8mb+a'
    # free dim (a, oj) == (mb, a', oj); channel 1024 -> (kt-major, p)
    peb = peb.reshape(9, 8, 128, 2, 128).transpose(0, 2, 1, 3, 4)
    pe_host = _f8(peb.reshape(9, 128, 4, 2, 2, 128) * SA)

    # W_dense row-permuted by rho, K-tile major [8, 128, 1024] fp16
    wd_perm = W_dense[rho, :]                                    # [1024, 1024]
    wd_host = np.ascontiguousarray(wd_perm.reshape(8, 128, D0)).astype(f16)
    # fp8 copy for the D matmul, DoubleRow pair layout
    # wd8[tp, p, i, nb, n] = SW8 * W_dense[rho[128*(2tp+i)+p], 512*nb+n]
    wd8_host = _f8(
        (wd_perm * SW8).reshape(4, 2, 128, 2, 512).transpose(0, 2, 1, 3, 4)
    )

    bd_host = (b_dense * SD).reshape(1, D0).astype(f16)

    in_maps = []
    for k in range(NC):
        ch = 512 * k + sigma
        # conv weights as DoubleRow matmul RHS, BN scale s folded in:
        # cw[dd, p, tp, i, oc] = SB * s[ch[oc]] * conv_w[dd, 128*(2tp+i)+p, ch[oc]]
        cws = conv_w[:, :, :, ch] * s_all[ch]          # [3,3,1024,512]
        cw = cws.reshape(9, 8, 128, 512).transpose(0, 2, 1, 3)
        cw_host = _f8(cw.reshape(9, 128, 4, 2, 512) * SB)
        # BN shift as a K=1 seed-matmul rhs row, in PSUM scale
        t2 = (t_all[ch] * SC).reshape(1, 512).astype(f16)
        # A_X^T for this core's row strip, rho-permuted, SBUF layout
        # [8b, 128part, 8kt*128j]: axt[b, p, 128*kt+j] = A_X^T[b, 128kt+p, j]
        Xs = X[:, 128 * k:128 * (k + 1), :]        # [8,128,1024]
        axt = Xs.reshape(B, 4, 32, 32, 32).transpose(0, 2, 4, 1, 3).reshape(B, 1024, 128)
        axt = axt[:, rho, :].reshape(B, 8, 128, 128).transpose(0, 2, 1, 3)
        axt = np.ascontiguousarray(axt.reshape(B, 128, 1024)).astype(f16)
        # W_emb row block transposed into lhsC layout, pre-scaled by SY:
        # wrows[p, 128*k2 + j] = SY * W_emb[128k + j, rho[128*k2 + p]]
        wr = W_emb[128 * k:128 * (k + 1), :][:, rho]        # [128j, 1024d0']
        wrows = np.ascontiguousarray(
            (wr.T * SY).reshape(8, 128, 128).transpose(1, 0, 2).reshape(128, 4, 2, 128)
        ).astype(f16)
        in_maps.append({
            "cw": cw_host, "pe": pe_host, "t2": t2, "axt": axt,
            "wrows": wrows, "wd": wd_host, "wd8": wd8_host, "bd": bd_host,
            "ones": np.ones((1, 128), np.float16),
        })
    return in_maps


# ---------------------------------------------------------------- device code

def _build():
    import concourse.tile as tile
    import concourse.mybir as mybir
    from concourse import bacc

    f32 = mybir.dt.float32
    f16 = mybir.dt.float16
    f8 = mybir.dt.float8e4
    Alu = mybir.AluOpType
    DR = mybir.MatmulPerfMode.DoubleRow

    nc = bacc.Bacc("TRN2", target_bir_lowering=False, debug=False)

    cw_d = nc.dram_tensor("cw", [9, 128, 4, 2, 512], f8, kind="ExternalInput").ap()
    pe_d = nc.dram_tensor("pe", [9, 128, 4, 2, 2, 128], f8, kind="ExternalInput").ap()
    t2_d = nc.dram_tensor("t2", [1, 512], f16, kind="ExternalInput").ap()
    axt_d = nc.dram_tensor("axt", [8, 128, 1024], f16, kind="ExternalInput").ap()
    wrows_d = nc.dram_tensor("wrows", [128, 4, 2, 128], f16, kind="ExternalInput").ap()
    wd_d = nc.dram_tensor("wd", [8, 128, 1024], f16, kind="ExternalInput").ap()
    wd8_d = nc.dram_tensor("wd8", [4, 128, 2, 2, 512], f8, kind="ExternalInput").ap()
    bd_d = nc.dram_tensor("bd", [1, 1024], f16, kind="ExternalInput").ap()
    ones_d = nc.dram_tensor("ones", [1, 128], f16, kind="ExternalInput").ap()
    out_d = nc.dram_tensor("out", [8, 128, 1024], f16, kind="ExternalOutput").ap()

    with tile.TileContext(nc) as tc:
        with (
            tc.tile_pool(name="kpool", bufs=1) as kpool,
            tc.tile_pool(name="zpool", bufs=2) as zpool,
            tc.tile_pool(name="opool", bufs=8) as opool,
        ):
            # persistent SBUF tensors
            pe_sb = kpool.tile([128, 9, 4, 2, 2, 128], f8, tag="pe")
            cw_sb = kpool.tile([128, 9, 4, 2, 512], f8, tag="cw")
            t2_sb = kpool.tile([1, 512], f16, tag="t2")
            Y2 = kpool.tile([128, 2, 512], f16, tag="Y2")   # 8*lrelu(BN(conv))
            lhsC8 = kpool.tile([128, 4, 2, 128], f8, tag="lhsC")
            drows = kpool.tile([128, 1024], f16, tag="drows")
            wrows_sb = kpool.tile([128, 4, 2, 128], f16, tag="wrows")
            bd_sb = kpool.tile([1, 1024], f16, tag="bd")
            ones_sb = kpool.tile([1, 128], f16, tag="ones")
            wd_sb = kpool.tile([128, 8, 1024], f16, tag="wd")
            wd8_sb = kpool.tile([128, 4, 2, 2, 512], f8, tag="wd8")
            axt_sb = kpool.tile([128, 8, 1024], f16, tag="axt")

            # ---------------- conv with BN folded into the PE:
            #   scale s folded into cw on host; shift t is a K=1 seed
            #   matmul (ones^T @ t2) accumulated first into each PSUM bank.
            # DoubleRow fp8: lhsT = pe [128ch, 2kt, 128m], rhs = cw
            # [128ch, 2kt, 512oc]; each MM contracts K=256.
            with tc.tile_pool(name="psA", bufs=1, space="PSUM") as psA, \
                 tc.tile_pool(name="psW", bufs=1, space="PSUM") as psW:
                y2ps = [psA.tile([128, 512], f32, tag=f"y2{mb}", name=f"y2ps{mb}")
                        for mb in range(2)]
                # t2/ones lead the sync ring (tiny) so the seeds run just
                # before the first conv MM; wrows/bd ride the scalar ring
                nc.sync.dma_start(t2_sb[:], t2_d[:])
                nc.sync.dma_start(ones_sb[:], ones_d[:])
                nc.scalar.dma_start(wrows_sb[:], wrows_d[:])
                nc.scalar.dma_start(bd_sb[:], bd_d[:])
                # PE warm-up on memset data (no DMA deps; gpsimd memsets run
                # earliest): HAM hits K=8/8 before the first data matmul
                wps = psW.tile([128, 512], f32, tag="wps")
                wsrc = kpool.tile([1, 128], f16, tag="wsrc")
                wpad = kpool.tile([1, 512], f16, tag="wpad")
                nc.gpsimd.memset(wsrc[:], 1.0)
                nc.gpsimd.memset(wpad[:], 0.0)
                for w in range(6):
                    nc.tensor.matmul(
                        wps[:], wsrc[:], wpad[:],
                        start=(w == 0), stop=(w == 5))
                for mb in range(2):
                    nc.tensor.matmul(
                        y2ps[mb][:], ones_sb[:], t2_sb[:],
                        start=True, stop=False)
                # conv data on the sync ring in consumption order; one DMA
                # per tensor per round (each dma_start costs ~0.6us of
                # engine issue time, so finer splits lose to issue rate)
                for dd in range(9):
                    if dd == 0:
                        # halves, so the first MMs start one transfer earlier
                        nc.sync.dma_start(pe_sb[:, 0, :2], pe_d[0][:, :2])
                        nc.sync.dma_start(cw_sb[:, 0, :2], cw_d[0][:, :2])
                        nc.sync.dma_start(pe_sb[:, 0, 2:], pe_d[0][:, 2:])
                        nc.sync.dma_start(cw_sb[:, 0, 2:], cw_d[0][:, 2:])
                    else:
                        nc.sync.dma_start(pe_sb[:, dd], pe_d[dd])
                        nc.sync.dma_start(cw_sb[:, dd], cw_d[dd])
                    for mb in range(2):
                        for tp in range(4):
                            nc.tensor.matmul(
                                y2ps[mb][:],
                                pe_sb[:, dd, tp, :, mb, :],
                                cw_sb[:, dd, tp, :, :],
                                start=False,
                                stop=(dd == 8 and tp == 3 and mb == 1),
                                perf_mode=DR)
                # dense-path loads, in consumption order; wd8 quarters are
                # interleaved into the wd stream so the D matmuls never
                # wait long enough to cross the 3.4us HAM re-throttle
                # window (a half-clock cliff for the rest of the phase)
                nc.sync.dma_start(axt_sb[:, 0], axt_d[0])
                nc.sync.dma_start(wd_sb[:, 0], wd_d[0])
                nc.sync.dma_start(axt_sb[:, 1], axt_d[1])
                nc.sync.dma_start(wd_sb[:, 1], wd_d[1])
                nc.sync.dma_start(wd_sb[:, 2], wd_d[2])
                for i in range(4):
                    nc.sync.dma_start(wd8_sb[:, i], wd8_d[i])
                    nc.sync.dma_start(wd_sb[:, 3 + i], wd_d[3 + i])
                nc.sync.dma_start(wd_sb[:, 7], wd_d[7])
                for b in range(2, 8):
                    nc.sync.dma_start(axt_sb[:, b], axt_d[b])

                # ---------------- Y2 = 8*lrelu(psum/4096): ACT copies PSUM
                # out with scale, DVE applies lrelu (one-PSUM-operand rule)
                for mb in range(2):
                    z = zpool.tile([128, 512], f16, tag="z")
                    nc.scalar.activation(
                        z[:], y2ps[mb][:],
                        mybir.ActivationFunctionType.Copy,
                        bias=0.0, scale=float(SY / SC))
                    nc.vector.scalar_tensor_tensor(
                        Y2[:, mb], z[:], ALPHA, z[:], Alu.mult, Alu.max)

            # ---------------- lhsC8 = fp8(Y2 + 8*W_emb^T) (free-dim slices
            # only; (uv, mb) grouping == DoubleRow k-pair grouping)
            for mb in range(2):
                nc.vector.tensor_tensor(
                    lhsC8[:, :, mb, :],
                    Y2[:, mb],
                    wrows_sb[:, :, mb, :], Alu.add)

            # ---------------- D rows (fp8 DoubleRow) then per-batch row
            # matmuls (fp16), kt-skewed over the first SKEW groups so the
            # PE follows the wd k-tile DMA stream without stalling.
            with tc.tile_pool(name="psD", bufs=1, space="PSUM") as psD, \
                 tc.tile_pool(name="psF", bufs=6, space="PSUM") as psF:

                groups = [(b, nb) for b in range(8) for nb in range(2)]
                fp = {}

                def emit_add(g):
                    b, nb = groups[g]
                    ot = opool.tile([128, 512], f16, tag="ot",
                                    name=f"ot{b}_{nb}")
                    nc.vector.tensor_tensor(
                        ot[:], fp[g][:],
                        drows[:, 512 * nb:512 * (nb + 1)], Alu.add)
                    nc.scalar.dma_start(
                        out_d[b][:, 512 * nb:512 * (nb + 1)], ot[:])

                def emit_mm(g, kt, add=True):
                    b, nb = groups[g]
                    if kt == 0:
                        fp[g] = psF.tile([128, 512], f32, tag="fp",
                                         name=f"fp{b}_{nb}")
                    nc.tensor.matmul(
                        fp[g][:],
                        axt_sb[:, b, 128 * kt:128 * (kt + 1)],
                        wd_sb[:, kt, 512 * nb:512 * (nb + 1)],
                        start=(kt == 0), stop=(kt == 7))
                    if kt == 7 and add:
                        emit_add(g)

                # b0/b1 groups kt-skewed: the PE follows the wd k-tile
                # stream right after the conv without waiting for all of wd.
                # Groups 0..5 hold the 6 psF banks until their adds flush
                # after drows (Vector queue is in-order; the adds depend on
                # it, and bank reuse by later groups must not precede D on
                # the PE queue).
                SKEW = 5
                for t in range(8 + SKEW - 1):
                    for g in range(SKEW):
                        kt = t - g
                        if 0 <= kt < 8:
                            emit_mm(g, kt, add=False)
                for g in range(SKEW, 6):
                    for kt in range(8):
                        emit_mm(g, kt, add=False)

                # D rows (fp8 DoubleRow); wd8 lands about now and the D
                # matmuls fill the axt3/axt4 DMA wait.
                dps = psD.tile([128, 1024], f32, tag="dps")
                for nb in range(2):
                    o = dps[:, 512 * nb:512 * (nb + 1)]
                    nc.tensor.matmul(
                        o, ones_sb[:], bd_sb[:, 512 * nb:512 * (nb + 1)],
                        start=True, stop=False)
                    for tp in range(4):
                        nc.tensor.matmul(
                            o, lhsC8[:, tp, :, :], wd8_sb[:, tp, :, nb, :],
                            start=False, stop=(tp == 3), perf_mode=DR)
                nc.vector.tensor_scalar(
                    drows[:], dps[:], float(1.0 / SD), None, Alu.mult)
                for g in range(6):
                    emit_add(g)

                for g in range(6, 16):
                    for kt in range(8):
                        emit_mm(g, kt)

    nc.compile()
    return nc


def get_nc():
    if "nc" not in _CACHE:
        _CACHE["nc"] = _build()
    return _CACHE["nc"]


# ---------------------------------------------------------------- entry points

def run(inputs, trace=False, **kwargs):
    from concourse.bass_utils import run_bass_kernel_spmd
    nc = get_nc()
    in_maps = host_prep(inputs)
    res = run_bass_kernel_spmd(nc, in_maps, list(range(NC)), trace=trace, **kwargs)
    out = np.empty((B, N0, D0), np.float32)
    for k in range(NC):
        out[:, 128 * k:128 * (k + 1), :] = res.results[k]["out"].astype(np.float32)
    return out, res


def kernel(**inputs):
    out, _ = run(inputs)
    return out
